# revision 21
# baseline (speedup 1.0000x reference)
"""Multi-head attention kernel for 8 Trainium2 NeuronCores.

Problem: nn_MultiHeadAttention_49246095016569
  q,k,v: [S=2048, B=2, E=512] f32; per-head projections Wq/Wk/Wv [64,64],
  output FC Wfc [512,512] + bfc [512].
  The reference reshapes [S,B,E] -> [B,H,S,D] with a PLAIN reshape, so each
  (b,h) pair is a contiguous [2048,64] chunk of the flattened input.  There
  are 16 chunks; each of the 8 cores handles 2 chunks, fully independently
  (no collectives).  Output rows [512*i, 512*(i+1)) of the flattened
  [4096,512] output come from core i.

Math per chunk c (qc,kc,vc = [2048,64] slices):
  khp = kc @ g_t            (g_t = Wk.T @ Wq folds both QK projections)
  S   = qc @ khp.T          (= Q @ K.T exactly, up to rounding)
  P   = exp(S/8)            (softmax without max-subtraction; |S/8| < ~6)
  A   = (P @ (vc @ Wv.T)) / P.sum(axis=1)
  out_rows = A.reshape(256,512) @ Wfc.T + bfc

On-chip layout: everything is computed transposed (S^T tiles = khpT.T @ qhT)
so that softmax sums come free via a ones-column appended to V', and the FC
contraction can slice A^T directly with stride-8 access patterns.
"""

import numpy as np

import concourse.bass as bass
import concourse.mybir as mybir
import concourse.tile as tile
from concourse import bacc
from concourse import bass_utils
from concourse.masks import make_identity

F32 = mybir.dt.float32
F32R = mybir.dt.float32r
BF16 = mybir.dt.bfloat16

S = 2048
D = 64
E = 512
NCORES = 8
CHUNKS_PER_CORE = 2
KT = S // 128  # 16 k-tiles of 128
QB = S // 512  # 4 q-blocks of 512

# dtype of the streaming matmul operands.  bf16: 1 row/cycle at any clock +
# FWL weight loads.  F32R: ~2.6e-4 rel err but ran at half clock in practice.
MM_DT = BF16
ACT_EXP = mybir.ActivationFunctionType.Exp
ACT_LN = mybir.ActivationFunctionType.Ln
ACT_COPY = mybir.ActivationFunctionType.Copy


def build_core_program():
    nc = bacc.Bacc(trn_type="TRN2")

    q_in = nc.dram_tensor("q_in", (CHUNKS_PER_CORE * S, D), MM_DT, kind="ExternalInput")
    k_in = nc.dram_tensor("k_in", (CHUNKS_PER_CORE * S, D), MM_DT, kind="ExternalInput")
    v_in = nc.dram_tensor("v_in", (CHUNKS_PER_CORE * S, D), MM_DT, kind="ExternalInput")
    g_t = nc.dram_tensor("g_t", (D, D), MM_DT, kind="ExternalInput")
    wv_t = nc.dram_tensor("wv_t", (D, D), MM_DT, kind="ExternalInput")
    wfc_t = nc.dram_tensor("wfc_t", (E, E), MM_DT, kind="ExternalInput")
    bias = nc.dram_tensor("bias", (1, E), F32, kind="ExternalInput")
    out = nc.dram_tensor("out", (CHUNKS_PER_CORE * 256, E), F32, kind="ExternalOutput")

    with tile.TileContext(nc) as tc:
        with (
            tc.tile_pool(name="consts", bufs=1) as consts,
            tc.tile_pool(name="raw", bufs=2) as raw_pool,
            tc.tile_pool(name="tp", bufs=2) as tp_pool,
            tc.tile_pool(name="pt", bufs=6) as pt_pool,
            tc.tile_pool(name="at", bufs=2) as at_pool,
            tc.tile_pool(name="outp", bufs=2) as out_pool,
            tc.tile_pool(name="npool", bufs=2) as npool,
            tc.tile_pool(name="ps_work", bufs=1, space="PSUM") as ps_work,
            tc.tile_pool(name="ps_score", bufs=2, space="PSUM") as ps_score,
            tc.tile_pool(name="ps_acc", bufs=2, space="PSUM") as ps_acc,
            tc.tile_pool(name="ps_fc", bufs=1, space="PSUM") as ps_fc,
        ):
            identity = consts.tile([128, 128], MM_DT)
            make_identity(nc, identity[:])

            g_sb = consts.tile([D, D], MM_DT)
            nc.sync.dma_start(g_sb[:], g_t[:])
            wv_sb = consts.tile([D, D], MM_DT)
            nc.sync.dma_start(wv_sb[:], wv_t[:])
            # Wfc.T as [64, 8, 512]: slice j = wfc_sb[:, j, :] (base partition 0)
            wfc_sb = consts.tile([D, 8, E], MM_DT)
            nc.sync.dma_start(
                wfc_sb[:], wfc_t[:].rearrange("(j d) e -> d j e", d=D)
            )

            bias_sb = consts.tile([1, E], F32)
            nc.sync.dma_start(bias_sb[:], bias[:])
            # broadcast bias to 128 partitions once via a K=1 outer product
            ones1 = consts.tile([1, 128], F32)
            nc.vector.memset(ones1[:], 1.0)
            bias_ps = ps_work.tile([128, E], F32, tag="work")
            nc.tensor.matmul(bias_ps[:], ones1[:], bias_sb[:], start=True, stop=True)
            bias_bc = consts.tile([128, E], F32)
            nc.vector.tensor_copy(bias_bc[:], bias_ps[:])
            ones64 = consts.tile([1, D], MM_DT)
            nc.vector.memset(ones64[:], 1.0)
            ones_col = consts.tile([128, KT, 1], F32)
            nc.vector.memset(ones_col[:], 1.0)

            # ---- HAM warm-up: ~8us of dense dependency-free PE work while
            # the first DMAs land.  The clock gate is bistable: entering the
            # attention loop at 1.2GHz keeps it at 1.2GHz; entering warm
            # (2.4GHz) sustains.
            warm_ps = ps_fc.tile([128, 512], MM_DT, tag="fc")
            for _ in range(72):
                nc.tensor.transpose(
                    warm_ps[:, 0:128], identity[:], identity[:]
                )

            for c in range(CHUNKS_PER_CORE):
                co = c * S

                # ---- load raw chunk as [128, 16, 64]: row p holds s = 128t+p
                q_raw3 = raw_pool.tile([128, KT, D], MM_DT, tag="q_raw")
                k_raw3 = raw_pool.tile([128, KT, D], MM_DT, tag="k_raw")
                v_raw3 = raw_pool.tile([128, KT, D], MM_DT, tag="v_raw")
                for dst3, srcd in ((q_raw3, q_in), (k_raw3, k_in), (v_raw3, v_in)):
                    for hl in range(2):
                        nc.sync.dma_start(
                            dst3[:, 8 * hl : 8 * (hl + 1), :],
                            srcd[
                                co + 1024 * hl : co + 1024 * (hl + 1), :
                            ].rearrange("(t p) d -> p t d", p=128),
                        )
                q_raw = q_raw3[:].rearrange("p t d -> p (t d)")
                k_raw = k_raw3[:].rearrange("p t d -> p (t d)")
                v_raw = v_raw3[:].rearrange("p t d -> p (t d)")

                # ---- PE-transpose q,k,v into [64, 2048] MM_DT (col = s)
                # paired: one [128,128] transpose covers s-tiles t=2g, 2g+1
                qhT = tp_pool.tile([D, S], MM_DT, tag="qhT")
                khT = tp_pool.tile([D, S], MM_DT, tag="khT")
                vhT = tp_pool.tile([D, S], MM_DT, tag="vhT")
                for rawt, dstT in ((q_raw, qhT), (k_raw, khT), (v_raw, vhT)):
                    # s = 256 g + 128 h + p
                    dv = dstT[:].rearrange("d (g h p) -> d h g p", g=8, h=2)
                    ps_t = ps_work.tile([128, 1024], MM_DT, tag="work")
                    for g in range(8):  # 8 bf16 paired transposes in one bank
                        nc.tensor.transpose(
                            ps_t[:, 128 * g : 128 * (g + 1)],
                            rawt[:, 128 * g : 128 * (g + 1)],
                            identity[:],
                        )
                    pv = ps_t[:].rearrange("x (g j) -> x g j", g=8)
                    nc.vector.tensor_copy(dv[:, 0], pv[0:D])
                    nc.vector.tensor_copy(dv[:, 1], pv[D : 2 * D])

                # ---- khp^T = g_t.T @ khT  (folded QK projection)
                khpT = tp_pool.tile([D, S], MM_DT, tag="khpT")
                for n in range(QB):
                    ps_p = ps_work.tile([D, 512], F32, tag="work")
                    nc.tensor.matmul(
                        ps_p[:],
                        g_sb[:],
                        khT[:, 512 * n : 512 * (n + 1)],
                        start=True,
                        stop=True,
                    )
                    nc.vector.tensor_copy(khpT[:, 512 * n : 512 * (n + 1)], ps_p[:])

                # ---- V' = vc @ Wv.T with ones column: [128, 16*65] MM_DT
                vp = raw_pool.tile([128, KT * (D + 1)], MM_DT, tag="vp")
                vp3 = vp[:].rearrange("p (kt x) -> p kt x", x=D + 1)
                nc.vector.tensor_copy(vp3[:, :, D : D + 1], ones_col[:])
                for half in range(2):  # 8 projections of N=64 per psum bank
                    ps_v = ps_work.tile([128, 512], F32, tag="work")
                    for m in range(8):
                        kt = 8 * half + m
                        nc.tensor.matmul(
                            ps_v[:, D * m : D * (m + 1)],
                            vhT[:, 128 * kt : 128 * (kt + 1)],
                            wv_sb[:],
                            start=True,
                            stop=True,
                        )
                    nc.vector.tensor_copy(
                        vp3[:, 8 * half : 8 * half + 8, 0:D],
                        ps_v[:].rearrange("p (m x) -> p m x", x=D),
                    )

                # ---- attention: two interleaved q-block chains (A, B)
                # score tiles hold TWO k-tiles -> one exp per [128,1024]
                atT = at_pool.tile([D, S], MM_DT, tag=f"at{c}")
                atv = atT[:].rearrange("d (m r j) -> d m j r", m=2, j=8)
                srows = npool.tile([1, QB, 512], F32, tag="srows")
                pcps = []
                for pair in range(QB // 2):
                    qoA = 1024 * pair
                    qoB = qoA + 512
                    pavA = ps_acc.tile([D + 1, 512], F32, tag="acc")
                    pavB = ps_acc.tile([D + 1, 512], F32, tag="acc")
                    for g in range(KT // 2):
                        pts = []
                        for qo in (qoA, qoB):
                            st = ps_score.tile([128, 1024], F32, tag="score")
                            for u in range(2):
                                kt = 2 * g + u
                                nc.tensor.matmul(
                                    st[:, 512 * u : 512 * (u + 1)],
                                    khpT[:, 128 * kt : 128 * (kt + 1)],
                                    qhT[:, qo : qo + 512],
                                    start=True,
                                    stop=True,
                                )
                            ptile = pt_pool.tile([128, 1024], MM_DT, tag="pt")
                            nc.scalar.activation(ptile[:], st[:], ACT_EXP, scale=0.125)
                            pts.append(ptile)
                        for pav, ptile in zip((pavA, pavB), pts):
                            for u in range(2):
                                kt = 2 * g + u
                                nc.tensor.matmul(
                                    pav[:],
                                    vp3[:, kt],
                                    ptile[:, 512 * u : 512 * (u + 1)],
                                    start=(kt == 0),
                                    stop=(kt == KT - 1),
                                )
                    # free both accumulation banks right away; stash the
                    # unnormalized A^T and the sums row in SBUF
                    for pav, qb in ((pavA, 2 * pair), (pavB, 2 * pair + 1)):
                        pcp = npool.tile([D + 1, 512], F32, tag=f"pcp{qb}")
                        nc.vector.tensor_copy(pcp[:], pav[:])
                        nc.vector.tensor_copy(
                            srows[:, qb, :], pcp[D : D + 1, :]
                        )
                        pcps.append(pcp)

                # ---- batched normalize: 1/s for all 4 q-blocks in two ACT
                # ops (DVE reciprocal is ~6.5ns/elem serial; ACT ln+exp is
                # table-cheap when batched per chunk)
                lns = npool.tile([1, QB * 512], F32, tag="lns")
                nc.scalar.activation(lns[:], srows[:].rearrange("o q x -> o (q x)"), ACT_LN)
                rs_all = npool.tile([1, QB, 512], MM_DT, tag="rs_all")
                nc.scalar.activation(
                    rs_all[:].rearrange("o q x -> o (q x)"), lns[:], ACT_EXP, scale=-1.0
                )
                for qb in range(QB):
                    rb_ps = ps_work.tile([D, 512], F32, tag="work")
                    nc.tensor.matmul(
                        rb_ps[:], ones64[:], rs_all[:, qb, :], start=True, stop=True
                    )
                    rb = pt_pool.tile([D, 512], F32, tag="rb")
                    nc.vector.tensor_copy(rb[:], rb_ps[:])
                    nc.vector.tensor_mul(
                        atT[:, 512 * qb : 512 * (qb + 1)], pcps[qb][0:D, :], rb[:]
                    )

                # ---- FC: out rows rr (128 per r-tile), 8 accumulating matmuls
                for half in range(2):
                    po = ps_fc.tile([128, E], F32, tag="fc")
                    for j in range(8):
                        nc.tensor.matmul(
                            po[:],
                            atv[:, half, j, :],
                            wfc_sb[:, j, :],
                            start=(j == 0),
                            stop=(j == 7),
                        )
                    ot = out_pool.tile([128, E], F32, tag="out")
                    nc.vector.tensor_add(ot[:], po[:], bias_bc[:])
                    nc.sync.dma_start(
                        out[256 * c + 128 * half : 256 * c + 128 * (half + 1), :],
                        ot[:],
                    )

    nc.compile()
    return nc


_NC_CACHE = None


def _get_nc():
    global _NC_CACHE
    if _NC_CACHE is None:
        _NC_CACHE = build_core_program()
    return _NC_CACHE


def make_in_maps(q, k, v, Wq, Wk, Wv, Wfc, bfc):
    import ml_dtypes

    bf16 = ml_dtypes.bfloat16
    q = np.ascontiguousarray(q, dtype=np.float32)
    k = np.ascontiguousarray(k, dtype=np.float32)
    v = np.ascontiguousarray(v, dtype=np.float32)
    g_t = (
        (np.asarray(Wk, np.float32).T @ np.asarray(Wq, np.float32))
        .astype(bf16)
    )
    wv_t = np.ascontiguousarray(np.asarray(Wv, np.float32).T.astype(bf16))
    wfc_t = np.ascontiguousarray(np.asarray(Wfc, np.float32).T.astype(bf16))
    bias = np.asarray(bfc, np.float32).reshape(1, E)

    qf = q.reshape(-1).astype(bf16)
    kf = k.reshape(-1).astype(bf16)
    vf = v.reshape(-1).astype(bf16)
    C = S * D
    in_maps = []
    for i in range(NCORES):
        lo = 2 * i * C
        hi = (2 * i + 2) * C
        in_maps.append(
            dict(
                q_in=np.ascontiguousarray(qf[lo:hi].reshape(2 * S, D)),
                k_in=np.ascontiguousarray(kf[lo:hi].reshape(2 * S, D)),
                v_in=np.ascontiguousarray(vf[lo:hi].reshape(2 * S, D)),
                g_t=g_t,
                wv_t=wv_t,
                wfc_t=wfc_t,
                bias=bias,
            )
        )
    return in_maps


def kernel(q, k, v, Wq, Wk, Wv, Wfc, bfc, _trace=False):
    nc = _get_nc()
    in_maps = make_in_maps(q, k, v, Wq, Wk, Wv, Wfc, bfc)
    res = bass_utils.run_bass_kernel_spmd(
        nc, in_maps, core_ids=list(range(NCORES)), trace=_trace
    )
    out = np.concatenate([res.results[i]["out"] for i in range(NCORES)], axis=0)
    kernel.last_exec_time_ns = res.exec_time_ns
    kernel.last_results = res
    return out.reshape(S, 2, E)


# revision 22
# speedup vs baseline: 1.1101x; 1.1101x over previous
"""Multi-head attention kernel for 8 Trainium2 NeuronCores.

Problem: nn_MultiHeadAttention_49246095016569
  q,k,v: [S=2048, B=2, E=512] f32; per-head projections Wq/Wk/Wv [64,64],
  output FC Wfc [512,512] + bfc [512].
  The reference reshapes [S,B,E] -> [B,H,S,D] with a PLAIN reshape, so each
  (b,h) pair is a contiguous [2048,64] chunk of the flattened input.  There
  are 16 chunks; each of the 8 cores handles 2 chunks, fully independently
  (no collectives).  Output rows [512*i, 512*(i+1)) of the flattened
  [4096,512] output come from core i.

Math per chunk c (qc,kc,vc = [2048,64] slices):
  khp = kc @ g_t            (g_t = Wk.T @ Wq folds both QK projections)
  S   = qc @ khp.T          (= Q @ K.T exactly, up to rounding)
  P   = exp(S/8)            (softmax without max-subtraction; |S/8| < ~6)
  A   = (P @ (vc @ Wv.T)) / P.sum(axis=1)
  out_rows = A.reshape(256,512) @ Wfc.T + bfc

On-chip layout: everything is computed transposed (S^T tiles = khpT.T @ qhT)
so that softmax sums come free via a ones-column appended to V', and the FC
contraction can slice A^T directly with stride-8 access patterns.
"""

import numpy as np

import concourse.bass as bass
import concourse.mybir as mybir
import concourse.tile as tile
from concourse import bacc
from concourse import bass_utils
from concourse.masks import make_identity

F32 = mybir.dt.float32
F32R = mybir.dt.float32r
BF16 = mybir.dt.bfloat16

S = 2048
D = 64
E = 512
NCORES = 8
CHUNKS_PER_CORE = 2
KT = S // 128  # 16 k-tiles of 128
QB = S // 512  # 4 q-blocks of 512

# dtype of the streaming matmul operands.  bf16: 1 row/cycle at any clock +
# FWL weight loads.  F32R: ~2.6e-4 rel err but ran at half clock in practice.
MM_DT = BF16
ACT_EXP = mybir.ActivationFunctionType.Exp
ACT_LN = mybir.ActivationFunctionType.Ln
ACT_COPY = mybir.ActivationFunctionType.Copy


def build_core_program():
    nc = bacc.Bacc(trn_type="TRN2")

    q_in = nc.dram_tensor("q_in", (CHUNKS_PER_CORE * S, D), MM_DT, kind="ExternalInput")
    k_in = nc.dram_tensor("k_in", (CHUNKS_PER_CORE * S, D), MM_DT, kind="ExternalInput")
    v_in = nc.dram_tensor("v_in", (CHUNKS_PER_CORE * S, D), MM_DT, kind="ExternalInput")
    g_t = nc.dram_tensor("g_t", (D, D), MM_DT, kind="ExternalInput")
    wv_t = nc.dram_tensor("wv_t", (D, D), MM_DT, kind="ExternalInput")
    wfc_t = nc.dram_tensor("wfc_t", (E, E), MM_DT, kind="ExternalInput")
    bias = nc.dram_tensor("bias", (1, E), F32, kind="ExternalInput")
    out = nc.dram_tensor("out", (CHUNKS_PER_CORE * 256, E), F32, kind="ExternalOutput")

    with tile.TileContext(nc) as tc:
        with (
            tc.tile_pool(name="consts", bufs=1) as consts,
            tc.tile_pool(name="raw", bufs=2) as raw_pool,
            tc.tile_pool(name="tp", bufs=2) as tp_pool,
            tc.tile_pool(name="pt", bufs=6) as pt_pool,
            tc.tile_pool(name="at", bufs=2) as at_pool,
            tc.tile_pool(name="outp", bufs=2) as out_pool,
            tc.tile_pool(name="npool", bufs=2) as npool,
            tc.tile_pool(name="ps_work", bufs=1, space="PSUM") as ps_work,
            tc.tile_pool(name="ps_score", bufs=2, space="PSUM") as ps_score,
            tc.tile_pool(name="ps_acc", bufs=2, space="PSUM") as ps_acc,
            tc.tile_pool(name="ps_fc", bufs=1, space="PSUM") as ps_fc,
        ):
            identity = consts.tile([128, 128], MM_DT)
            make_identity(nc, identity[:])

            g_sb = consts.tile([D, D], MM_DT)
            nc.sync.dma_start(g_sb[:], g_t[:])
            wv_sb = consts.tile([D, D], MM_DT)
            nc.sync.dma_start(wv_sb[:], wv_t[:])
            # Wfc.T as [64, 8, 512]: slice j = wfc_sb[:, j, :] (base partition 0)
            wfc_sb = consts.tile([D, 8, E], MM_DT)
            nc.sync.dma_start(
                wfc_sb[:], wfc_t[:].rearrange("(j d) e -> d j e", d=D)
            )

            bias_sb = consts.tile([1, E], F32)
            nc.sync.dma_start(bias_sb[:], bias[:])
            # broadcast bias to 128 partitions once via a K=1 outer product
            ones1 = consts.tile([1, 128], F32)
            nc.vector.memset(ones1[:], 1.0)
            bias_ps = ps_work.tile([128, E], F32, tag="work")
            nc.tensor.matmul(bias_ps[:], ones1[:], bias_sb[:], start=True, stop=True)
            bias_bc = consts.tile([128, E], F32)
            nc.vector.tensor_copy(bias_bc[:], bias_ps[:])
            ones64 = consts.tile([1, D], MM_DT)
            nc.vector.memset(ones64[:], 1.0)
            ones_col = consts.tile([128, KT, 1], F32)
            nc.vector.memset(ones_col[:], 1.0)

            # ---- HAM warm-up: ~8us of dense dependency-free PE work while
            # the first DMAs land.  The clock gate is bistable: entering the
            # attention loop at 1.2GHz keeps it at 1.2GHz; entering warm
            # (2.4GHz) sustains.
            warm_ps = ps_fc.tile([128, 512], MM_DT, tag="fc")
            for _ in range(160):
                nc.tensor.transpose(
                    warm_ps[:, 0:128], identity[:], identity[:]
                )

            def emit_prep(c):
                co = c * S
                # load raw chunk as [128, 16, 64]: row p holds s = 128t+p
                q_raw3 = raw_pool.tile([128, KT, D], MM_DT, tag="q_raw")
                k_raw3 = raw_pool.tile([128, KT, D], MM_DT, tag="k_raw")
                v_raw3 = raw_pool.tile([128, KT, D], MM_DT, tag="v_raw")
                for dst3, srcd in ((q_raw3, q_in), (k_raw3, k_in), (v_raw3, v_in)):
                    for hl in range(2):
                        nc.sync.dma_start(
                            dst3[:, 8 * hl : 8 * (hl + 1), :],
                            srcd[
                                co + 1024 * hl : co + 1024 * (hl + 1), :
                            ].rearrange("(t p) d -> p t d", p=128),
                        )
                q_raw = q_raw3[:].rearrange("p t d -> p (t d)")
                k_raw = k_raw3[:].rearrange("p t d -> p (t d)")
                v_raw = v_raw3[:].rearrange("p t d -> p (t d)")

                # PE-transpose q,k,v into [64, 2048] MM_DT (col = s)
                # paired: one [128,128] transpose covers s-tiles t=2g, 2g+1
                qhT = tp_pool.tile([D, S], MM_DT, tag="qhT")
                khT = tp_pool.tile([D, S], MM_DT, tag="khT")
                vhT = tp_pool.tile([D, S], MM_DT, tag="vhT")
                for rawt, dstT in ((q_raw, qhT), (k_raw, khT), (v_raw, vhT)):
                    # s = 256 g + 128 h + p
                    dv = dstT[:].rearrange("d (g h p) -> d h g p", g=8, h=2)
                    ps_t = ps_work.tile([128, 1024], MM_DT, tag="work")
                    for g in range(8):  # 8 bf16 paired transposes in one bank
                        nc.tensor.transpose(
                            ps_t[:, 128 * g : 128 * (g + 1)],
                            rawt[:, 128 * g : 128 * (g + 1)],
                            identity[:],
                        )
                    pv = ps_t[:].rearrange("x (g j) -> x g j", g=8)
                    nc.vector.tensor_copy(dv[:, 0], pv[0:D])
                    nc.vector.tensor_copy(dv[:, 1], pv[D : 2 * D])

                # khp^T = g_t.T @ khT  (folded QK projection)
                khpT = tp_pool.tile([D, S], MM_DT, tag="khpT")
                for n in range(QB):
                    ps_p = ps_work.tile([D, 512], F32, tag="work")
                    nc.tensor.matmul(
                        ps_p[:],
                        g_sb[:],
                        khT[:, 512 * n : 512 * (n + 1)],
                        start=True,
                        stop=True,
                    )
                    nc.vector.tensor_copy(khpT[:, 512 * n : 512 * (n + 1)], ps_p[:])

                # V' = vc @ Wv.T with ones column: [128, 16*65] MM_DT
                vp = raw_pool.tile([128, KT * (D + 1)], MM_DT, tag="vp")
                vp3 = vp[:].rearrange("p (kt x) -> p kt x", x=D + 1)
                nc.vector.tensor_copy(vp3[:, :, D : D + 1], ones_col[:])
                for half in range(2):  # 8 projections of N=64 per psum bank
                    ps_v = ps_work.tile([128, 512], F32, tag="work")
                    for m in range(8):
                        kt = 8 * half + m
                        nc.tensor.matmul(
                            ps_v[:, D * m : D * (m + 1)],
                            vhT[:, 128 * kt : 128 * (kt + 1)],
                            wv_sb[:],
                            start=True,
                            stop=True,
                        )
                    nc.vector.tensor_copy(
                        vp3[:, 8 * half : 8 * half + 8, 0:D],
                        ps_v[:].rearrange("p (m x) -> p m x", x=D),
                    )
                return qhT, khpT, vp3

            def emit_attention(c, qhT, khpT, vp3):
                # two interleaved q-block chains (A, B) per pair phase;
                # score tiles hold TWO k-tiles -> one exp per [128,1024]
                atT = at_pool.tile([D, S], MM_DT, tag=f"at{c}")
                srows = npool.tile([1, QB, 512], F32, tag="srows")
                pcps = []
                for pair in range(QB // 2):
                    qoA = 1024 * pair
                    qoB = qoA + 512
                    pavA = ps_acc.tile([D + 1, 512], F32, tag="acc")
                    pavB = ps_acc.tile([D + 1, 512], F32, tag="acc")
                    for g in range(KT // 2):
                        pts = []
                        for qo in (qoA, qoB):
                            st = ps_score.tile([128, 1024], F32, tag="score")
                            for u in range(2):
                                kt = 2 * g + u
                                nc.tensor.matmul(
                                    st[:, 512 * u : 512 * (u + 1)],
                                    khpT[:, 128 * kt : 128 * (kt + 1)],
                                    qhT[:, qo : qo + 512],
                                    start=True,
                                    stop=True,
                                )
                            ptile = pt_pool.tile([128, 1024], MM_DT, tag="pt")
                            nc.scalar.activation(ptile[:], st[:], ACT_EXP, scale=0.125)
                            pts.append(ptile)
                        for pav, ptile in zip((pavA, pavB), pts):
                            for u in range(2):
                                kt = 2 * g + u
                                nc.tensor.matmul(
                                    pav[:],
                                    vp3[:, kt],
                                    ptile[:, 512 * u : 512 * (u + 1)],
                                    start=(kt == 0),
                                    stop=(kt == KT - 1),
                                )
                    # free both accumulation banks right away; stash the
                    # unnormalized A^T and the sums row in SBUF
                    for pav, qb in ((pavA, 2 * pair), (pavB, 2 * pair + 1)):
                        pcp = npool.tile([D + 1, 512], F32, tag=f"pcp{qb}")
                        nc.vector.tensor_copy(pcp[:], pav[:])
                        nc.vector.tensor_copy(srows[:, qb, :], pcp[D : D + 1, :])
                        pcps.append(pcp)
                return atT, srows, pcps

            def emit_tail(c, atT, srows, pcps):
                # batched normalize: 1/s for all 4 q-blocks in two ACT ops
                # (DVE reciprocal is ~6.5ns/elem serial; ACT ln+exp is
                # table-cheap when batched per chunk)
                atv = atT[:].rearrange("d (m r j) -> d m j r", m=2, j=8)
                lns = npool.tile([1, QB * 512], F32, tag="lns")
                nc.scalar.activation(
                    lns[:], srows[:].rearrange("o q x -> o (q x)"), ACT_LN
                )
                rs_all = npool.tile([1, QB, 512], MM_DT, tag="rs_all")
                nc.scalar.activation(
                    rs_all[:].rearrange("o q x -> o (q x)"),
                    lns[:],
                    ACT_EXP,
                    scale=-1.0,
                )
                for qb in range(QB):
                    rb_ps = ps_work.tile([D, 512], F32, tag="work")
                    nc.tensor.matmul(
                        rb_ps[:], ones64[:], rs_all[:, qb, :], start=True, stop=True
                    )
                    rb = pt_pool.tile([D, 512], F32, tag="rb")
                    nc.vector.tensor_copy(rb[:], rb_ps[:])
                    nc.vector.tensor_mul(
                        atT[:, 512 * qb : 512 * (qb + 1)], pcps[qb][0:D, :], rb[:]
                    )

                # FC: out rows rr (128 per r-tile), 8 accumulating matmuls
                for half in range(2):
                    po = ps_fc.tile([128, E], F32, tag="fc")
                    for j in range(8):
                        nc.tensor.matmul(
                            po[:],
                            atv[:, half, j, :],
                            wfc_sb[:, j, :],
                            start=(j == 0),
                            stop=(j == 7),
                        )
                    ot = out_pool.tile([128, E], F32, tag="out")
                    nc.vector.tensor_add(ot[:], po[:], bias_bc[:])
                    nc.sync.dma_start(
                        out[256 * c + 128 * half : 256 * c + 128 * (half + 1), :],
                        ot[:],
                    )

            # software-pipeline the chunks: chunk c+1's prep is emitted
            # BEFORE chunk c's tail so the PE queue never head-of-line
            # blocks on the (ACT/DVE-bound) normalize tail
            t0 = emit_prep(0)
            a0 = emit_attention(0, *t0)
            t1 = emit_prep(1)
            emit_tail(0, *a0)
            a1 = emit_attention(1, *t1)
            emit_tail(1, *a1)

    nc.compile()
    return nc


_NC_CACHE = None


def _get_nc():
    global _NC_CACHE
    if _NC_CACHE is None:
        _NC_CACHE = build_core_program()
    return _NC_CACHE


def make_in_maps(q, k, v, Wq, Wk, Wv, Wfc, bfc):
    import ml_dtypes

    bf16 = ml_dtypes.bfloat16
    q = np.ascontiguousarray(q, dtype=np.float32)
    k = np.ascontiguousarray(k, dtype=np.float32)
    v = np.ascontiguousarray(v, dtype=np.float32)
    g_t = (
        (np.asarray(Wk, np.float32).T @ np.asarray(Wq, np.float32))
        .astype(bf16)
    )
    wv_t = np.ascontiguousarray(np.asarray(Wv, np.float32).T.astype(bf16))
    wfc_t = np.ascontiguousarray(np.asarray(Wfc, np.float32).T.astype(bf16))
    bias = np.asarray(bfc, np.float32).reshape(1, E)

    qf = q.reshape(-1).astype(bf16)
    kf = k.reshape(-1).astype(bf16)
    vf = v.reshape(-1).astype(bf16)
    C = S * D
    in_maps = []
    for i in range(NCORES):
        lo = 2 * i * C
        hi = (2 * i + 2) * C
        in_maps.append(
            dict(
                q_in=np.ascontiguousarray(qf[lo:hi].reshape(2 * S, D)),
                k_in=np.ascontiguousarray(kf[lo:hi].reshape(2 * S, D)),
                v_in=np.ascontiguousarray(vf[lo:hi].reshape(2 * S, D)),
                g_t=g_t,
                wv_t=wv_t,
                wfc_t=wfc_t,
                bias=bias,
            )
        )
    return in_maps


def kernel(q, k, v, Wq, Wk, Wv, Wfc, bfc, _trace=False):
    nc = _get_nc()
    in_maps = make_in_maps(q, k, v, Wq, Wk, Wv, Wfc, bfc)
    res = bass_utils.run_bass_kernel_spmd(
        nc, in_maps, core_ids=list(range(NCORES)), trace=_trace
    )
    out = np.concatenate([res.results[i]["out"] for i in range(NCORES)], axis=0)
    kernel.last_exec_time_ns = res.exec_time_ns
    kernel.last_results = res
    return out.reshape(S, 2, E)
